# revision 3
# baseline (speedup 1.0000x reference)
"""MultiLabelSoftMarginLoss (logits=True path) on 8 Trainium2 NeuronCores.

Math (per sample b, C classes, K labels t_bk, ls = log_sigmoid, sp = softplus):
  pos_mean_b = (1/K) sum_k ls(x[b, t_bk])            = (1/K)(sum_k g_bk - sum_k sp(g_bk))
  neg_mean_b = [sum_c ls(-x_bc) - sum_{unique u} ls(-x_bu)] / (C - n_unique_b)
             = [-sum_c sp(x_bc) + sum_k w_bk sp(g_bk)] / (C - sum_k w_bk)
  where g_bk = x[b, t_bk] (indirect-DMA gather) and w_bk = 1 iff first
  occurrence of the label value within row b (dedup weights).
  loss = -mean_b(pos_mean_b + neg_mean_b)

Data-parallel: 2048 rows sharded 256/core; each core streams its
[256, 50257] f32 shard once (DMA-bound, ~51.5 MB), fused
softplus+row-sum on the ACT engine, gather+dedup corrections on
DVE/GPSIMD, emits 256 per-row losses; host sums and negates.
"""

import numpy as np

import concourse.bacc as bacc
import concourse.bass as bass
import concourse.mybir as mybir
import concourse.tile as tile
from concourse.bass_utils import run_bass_kernel_spmd

B, C, K = 2048, 50257, 20
NCORES = 8
RPC = B // NCORES  # rows per core
P = 128
NBLK = RPC // P  # row blocks of 128 partitions per core
CHUNK = 4096
NCHUNK = -(-C // CHUNK)

F32 = mybir.dt.float32
I32 = mybir.dt.int32
AF = mybir.ActivationFunctionType
ALU = mybir.AluOpType

# Set False to compute softplus as Ln(Exp(x) + 1) (two ACT passes, one
# shared table set) if the native Softplus activation is unsupported.
USE_NATIVE_SOFTPLUS = False

_CACHE = {}


def _build():
    nc = bacc.Bacc(
        "TRN2", target_bir_lowering=False, debug=False, num_devices=NCORES
    )
    x = nc.dram_tensor("x", [RPC, C], F32, kind="ExternalInput").ap()
    t = nc.dram_tensor("t", [RPC, K], I32, kind="ExternalInput").ap()
    out = nc.dram_tensor("out", [NBLK, P], F32, kind="ExternalOutput").ap()

    with tile.TileContext(nc) as tc:
        with (
            tc.tile_pool(name="xpool", bufs=4) as xpool,
            tc.tile_pool(name="scr", bufs=2) as scr,
            tc.tile_pool(name="small", bufs=2) as small,
        ):
            for blk in range(NBLK):
                rows = slice(blk * P, (blk + 1) * P)

                # --- bulk: sum_c softplus(x) per row, streamed in chunks ---
                parts = small.tile([P, NCHUNK], F32, tag="parts")
                for j in range(NCHUNK):
                    c0 = j * CHUNK
                    cw = min(C, c0 + CHUNK) - c0
                    xt = xpool.tile([P, CHUNK], F32, tag="xt")
                    nc.sync.dma_start(out=xt[:, :cw], in_=x[rows, c0 : c0 + cw])
                    if USE_NATIVE_SOFTPLUS:
                        sp = scr.tile([P, CHUNK], F32, tag="sp")
                        nc.scalar.activation(
                            sp[:, :cw],
                            xt[:, :cw],
                            AF.Softplus,
                            accum_out=parts[:, j : j + 1],
                        )
                    else:
                        ex = scr.tile([P, CHUNK], F32, tag="sp")
                        nc.scalar.activation(ex[:, :cw], xt[:, :cw], AF.Exp)
                        nc.scalar.activation(
                            ex[:, :cw],
                            ex[:, :cw],
                            AF.Ln,
                            bias=1.0,
                            accum_out=parts[:, j : j + 1],
                        )

                # --- gather g = x[row, t[row, k]] via indirect DMA ---
                tt = small.tile([P, K], I32, tag="tt")
                nc.sync.dma_start(out=tt[:], in_=t[rows, :])
                rowbase = small.tile([P, 1], I32, tag="rowbase")
                nc.gpsimd.iota(
                    rowbase[:], pattern=[[0, 1]], base=blk * P * C,
                    channel_multiplier=C,
                )
                offs = small.tile([P, K], I32, tag="offs")
                nc.vector.tensor_tensor(
                    out=offs[:], in0=tt[:],
                    in1=rowbase[:].to_broadcast([P, K]), op=ALU.add,
                )
                g = small.tile([P, K], F32, tag="g")
                nc.gpsimd.indirect_dma_start(
                    out=g[:],
                    out_offset=None,
                    in_=x[:, :],
                    in_offset=bass.IndirectOffsetOnAxis(ap=offs[:], axis=1),
                )

                # --- softplus(g) and row sums ---
                spg = small.tile([P, K], F32, tag="spg")
                spg_sum = small.tile([P, 1], F32, tag="spg_sum")
                if USE_NATIVE_SOFTPLUS:
                    nc.scalar.activation(
                        spg[:], g[:], AF.Softplus, accum_out=spg_sum[:]
                    )
                else:
                    nc.scalar.activation(spg[:], g[:], AF.Exp)
                    nc.scalar.activation(
                        spg[:], spg[:], AF.Ln, bias=1.0, accum_out=spg_sum[:]
                    )
                g_sum = small.tile([P, 1], F32, tag="g_sum")
                nc.vector.reduce_sum(
                    out=g_sum[:], in_=g[:], axis=mybir.AxisListType.X
                )

                # --- dedup weights: w_bk = 1 iff first occurrence in row ---
                tf = small.tile([P, K], F32, tag="tf")
                nc.vector.tensor_copy(out=tf[:], in_=tt[:])
                dup = small.tile([P, K], F32, tag="dup")
                nc.vector.memset(dup[:, 0:1], 0.0)
                eq = small.tile([P, K], F32, tag="eq")
                for k in range(1, K):
                    nc.vector.tensor_scalar(
                        out=eq[:, :k], in0=tf[:, :k], scalar1=tf[:, k : k + 1],
                        scalar2=None, op0=ALU.is_equal,
                    )
                    nc.vector.reduce_max(
                        out=dup[:, k : k + 1], in_=eq[:, :k],
                        axis=mybir.AxisListType.X,
                    )
                w = small.tile([P, K], F32, tag="w")
                nc.vector.tensor_scalar(
                    out=w[:], in0=dup[:], scalar1=-1.0, scalar2=1.0,
                    op0=ALU.mult, op1=ALU.add,
                )
                u = small.tile([P, 1], F32, tag="u")
                nc.vector.reduce_sum(
                    out=u[:], in_=w[:], axis=mybir.AxisListType.X
                )
                wspg = small.tile([P, K], F32, tag="wspg")
                nc.vector.tensor_tensor(
                    out=wspg[:], in0=w[:], in1=spg[:], op=ALU.mult
                )
                corr = small.tile([P, 1], F32, tag="corr")
                nc.vector.reduce_sum(
                    out=corr[:], in_=wspg[:], axis=mybir.AxisListType.X
                )

                # --- combine ---
                total = small.tile([P, 1], F32, tag="total")
                nc.vector.reduce_sum(
                    out=total[:], in_=parts[:], axis=mybir.AxisListType.X
                )
                denom = small.tile([P, 1], F32, tag="denom")
                nc.vector.tensor_scalar(
                    out=denom[:], in0=u[:], scalar1=-1.0, scalar2=float(C),
                    op0=ALU.mult, op1=ALU.add,
                )
                recip = small.tile([P, 1], F32, tag="recip")
                nc.vector.reciprocal(out=recip[:], in_=denom[:])
                negm = small.tile([P, 1], F32, tag="negm")
                nc.vector.tensor_sub(out=negm[:], in0=corr[:], in1=total[:])
                nc.vector.tensor_mul(out=negm[:], in0=negm[:], in1=recip[:])
                posm = small.tile([P, 1], F32, tag="posm")
                nc.vector.tensor_sub(out=posm[:], in0=g_sum[:], in1=spg_sum[:])
                nc.vector.tensor_scalar(
                    out=posm[:], in0=posm[:], scalar1=1.0 / K, scalar2=None,
                    op0=ALU.mult,
                )
                loss = small.tile([P, 1], F32, tag="loss")
                nc.vector.tensor_add(out=loss[:], in0=posm[:], in1=negm[:])
                nc.sync.dma_start(out=out[blk, :, None], in_=loss[:])

    nc.compile()
    return nc


def kernel(inputs: np.ndarray, targets: np.ndarray, _trace: bool = False):
    inputs = np.ascontiguousarray(inputs, dtype=np.float32)
    targets = np.ascontiguousarray(targets, dtype=np.int32)
    assert inputs.shape == (B, C) and targets.shape == (B, K)

    if "nc" not in _CACHE:
        _CACHE["nc"] = _build()
    nc = _CACHE["nc"]

    in_maps = [
        {
            "x": inputs[i * RPC : (i + 1) * RPC],
            "t": targets[i * RPC : (i + 1) * RPC],
        }
        for i in range(NCORES)
    ]
    res = run_bass_kernel_spmd(
        nc, in_maps, core_ids=list(range(NCORES)), trace=_trace
    )
    _CACHE["last_results"] = res

    per_row = np.concatenate(
        [res.results[i]["out"].reshape(-1) for i in range(NCORES)]
    )
    return np.float32(-np.mean(per_row, dtype=np.float64))
